# revision 40
# baseline (speedup 1.0000x reference)
"""Causal self-attention (B=2, T=2048, C=768, H=12) on 8 Trainium2 cores.

Sharding: 24 (batch, head) pairs / 8 cores = 3 heads per core.
core c -> batch b = c // 4, heads [3g, 3g+3) with g = c % 4.

All matmuls in bf16 (fp32 PSUM accumulation). fro-rel error ~1e-3,
well under the 2e-2 gate, and 2x the PE column rate of fp32r while
keeping the HAM clock-gate warm (fp32 streams let it re-throttle).

Per-core device program:
  qkT   = Wqk_local^T @ xT_aug        [384, T]  (q cols pre-scaled 1/8;
          q bias folded in as a 7th contraction chunk: xT row 768 = 1,
          wqk row 768 = bias; k bias dropped: softmax-invariant)
  V     = x_b @ Wv_local              [T, 192]  (v bias folded on host)
  per head h, per 512-wide q-chunk, kb blocks PAIRED (even kb on PE
  row-strip 0:64, odd kb on strip 64:128, concurrent via tile_position
  auto-derivation -> ~2x score matmul throughput; q/k live at BOTH
  partition offsets, built by SBUF->SBUF DMA relocation):
    scoresT pair -> one 2-bank PSUM tile [128, 1024]
    expT = exp(pair)                  (single ACT call per full pair;
                                       diagonal blocks trimmed+masked)
    y_augT[[v;1], q] += V_aug^T @ expT  (ones col -> softmax denom)
    yT_h = y_augT[y rows] * recip(denom)  (DVE recip, gpsimd
                                       partition_broadcast, DVE mul)
  out_partial = Y_local @ Wp_local    [T, 768] bf16 out (one q-chunk
                                       late, PE gap-filler)

Host: out[b] = sum of the 4 partials + (b_proj + b_v @ W_proj).

qkT chunk layout [128, 6, T] (matmul operands must sit on the same
base partition as their PE row-strip, so every head's q and k exist at
offset 0 AND 64):
  c0=[q0|q1] c1=[k0|k2->no: k0|k1] c2=[q2|k2]   (projection chains)
  c3=[q1|q0] c4=[k1|k0] c5=[k2|q2]              (DMA relocations)
V_aug per-kb free layout [65 | 128 | 128]:
  h0: [v_h0, 1] (denom row 64, y rows 0:64)
  h1: [1, 0*63, v_h1] (denom row 0, y rows 64:128)
  h2: [1, 0*63, v_h2] (denom row 0, y rows 64:128)
yT layout [128, 2, T]: h0 -> (0:64, 0), h1 -> (64:128, 0),
  h2 -> (64:128, 1); out-proj: K=128 chunk0 matmul + K=64 chunk1
  matmul (rows 64:128) per 384-wide half.
"""

import numpy as np
import ml_dtypes

import concourse.bass as bass
import concourse.mybir as mybir
import concourse.tile as tile
from concourse import bacc
from concourse import bass_utils

P = 128
D = 64          # head dim
HPC = 3         # heads per core
C = 768
CK = 6          # full contraction chunks of x
CKA = 7         # + bias chunk
QK = 384        # q+k cols per core
NH = 12
B = 2
N_CORES = 8
F32 = mybir.dt.float32
BF16 = mybir.dt.bfloat16
BF = ml_dtypes.bfloat16

# q/k positions: (partition offset, chunk) at lo (strip 0) and hi (strip 64)
Q_LO = [(0, 0), (0, 3), (0, 2)]
Q_HI = [(64, 3), (64, 0), (64, 5)]
K_LO = [(0, 1), (0, 4), (0, 5)]
K_HI = [(64, 4), (64, 1), (64, 2)]
# V_aug free-layout per head: (lhsT start, lhsT width, denom row, y row0)
V_SLICE = [(0, 65, 64, 0), (65, 128, 0, 64), (193, 128, 0, 64)]
VW = 321
# yT destination (row0, chunk) per head
Y_POS = [(0, 0), (64, 0), (64, 1)]


def build_nc(T=2048, QCW=512, debug_dump=False, debug_stages=7):
    """Build the per-core Bass program. T = sequence length, QCW = q-chunk."""
    assert T % QCW == 0 and QCW % P == 0 and T % 512 == 0
    NQC = T // QCW
    NTB = T // P
    NPH = C // 2  # 384, out-proj free-dim half

    nc = bacc.Bacc("TRN2", target_bir_lowering=False, debug=False,
                   num_devices=N_CORES)
    xT = nc.dram_tensor("xT", [C, T], BF16, kind="ExternalInput").ap()
    wqk = nc.dram_tensor("wqk", [CKA * P, QK], BF16, kind="ExternalInput").ap()
    wv = nc.dram_tensor("wv", [C, HPC * D], BF16, kind="ExternalInput").ap()
    wp = nc.dram_tensor("wp", [192, C], BF16, kind="ExternalInput").ap()
    out = nc.dram_tensor("out", [T, C], BF16, kind="ExternalOutput").ap()
    if debug_dump:
        qkT_dump = nc.dram_tensor("qkT_dump", [P, 6, T], BF16,
                                  kind="ExternalOutput").ap()
        v_dump = nc.dram_tensor("v_dump", [P, NTB, VW], BF16,
                                kind="ExternalOutput").ap()
        yT_dump = nc.dram_tensor("yT_dump", [P, 2, T], BF16,
                                 kind="ExternalOutput").ap()
        mask_dump = nc.dram_tensor("mask_dump", [P, 4, QCW], BF16,
                                   kind="ExternalOutput").ap()
        xT_dump = nc.dram_tensor("xT_dump", [P, CKA, T], BF16,
                                 kind="ExternalOutput").ap()

    Exp = mybir.ActivationFunctionType.Exp

    with tile.TileContext(nc) as tc:
        with (
            tc.tile_pool(name="const", bufs=1) as const,
            tc.tile_pool(name="work", bufs=6) as work,
            tc.tile_pool(name="small", bufs=3) as small,
            tc.tile_pool(name="outp", bufs=3) as outp,
            tc.tile_pool(name="ps_mm", bufs=2, space="PSUM") as ps_mm,
            tc.tile_pool(name="ps_s", bufs=2, space="PSUM") as ps_s,
            tc.tile_pool(name="ps_y", bufs=2, space="PSUM") as ps_y_pool,
        ):
            xT_sb = const.tile([P, CKA, T], BF16, tag="xT")
            wqk_sb = const.tile([P, CKA, QK], BF16, tag="wqk")
            wv_sb = const.tile([P, CK, HPC * D], BF16, tag="wv")
            wp_sb = const.tile([P, 2, C], BF16, tag="wp")
            qkT_sb = const.tile([P, 6, T], BF16, tag="qkT")
            v_sb = const.tile([P, NTB, VW], BF16, tag="v")
            yT_sb = const.tile([P, 2, T], BF16, tag="yT")
            zb_sb = const.tile([P, 1], F32, tag="zb")
            ones1_sb = const.tile([1, P], BF16, tag="ones1")
            mask_sb = const.tile([P, 4, QCW], BF16, tag="mask")

            # ---- loads (wqk + xT tj0 first so chains start early) ----
            in_engs = [nc.sync, nc.gpsimd, nc.scalar]
            di = 0
            for kc in range(CKA):
                in_engs[di % 3].dma_start(
                    wqk_sb[:, kc, :], wqk[kc * P:(kc + 1) * P, :])
                di += 1
                if kc < CK:
                    in_engs[di % 3].dma_start(
                        xT_sb[:, kc, 0:512], xT[kc * P:(kc + 1) * P, 0:512])
                    di += 1
            nc.gpsimd.dma_start(
                wv_sb[:], wv.rearrange("(kc p) m -> p kc m", p=P))
            for tj in range(1, T // 512):
                for kc in range(CK):
                    # scalar queue only for early slices: ACT gets busy
                    eng = in_engs[di % 3] if tj == 1 else in_engs[di % 2]
                    eng.dma_start(
                        xT_sb[:, kc, tj * 512:(tj + 1) * 512],
                        xT[kc * P:(kc + 1) * P, tj * 512:(tj + 1) * 512])
                    di += 1
            # wp needed only from attn(1) on: queue behind the xT loads
            nc.gpsimd.dma_start(wp_sb[:, 0, :], wp[0:128, :])
            nc.gpsimd.dma_start(wp_sb[64:128, 1, :], wp[128:192, :])

            # ---- constants ----
            st = const.tile([P, 4], F32, tag="st")
            nc.gpsimd.memset(st[:, 0:2], 1.0)
            nc.gpsimd.memset(st[:, 2:4], 0.0)
            # bias chunk: row 0 (= x row 768) is all-ones, rest zero
            nc.gpsimd.memset(xT_sb[:, CKA - 1, :], 0.0)
            nc.gpsimd.memset(xT_sb[0:1, CKA - 1, :], 1.0)
            nc.gpsimd.memset(zb_sb[:], 0.0)
            nc.gpsimd.memset(ones1_sb[:], 1.0)
            # V_aug constant cols: h0 ones col 64, h1 denom col 65 +
            # zeros 66:129, h2 denom col 193 + zeros 194:257
            nc.vector.tensor_copy(v_sb[:, :, 64:66],
                                  st[:, None, 0:2].to_broadcast((P, NTB, 2)))
            nc.vector.tensor_copy(v_sb[:, :, 193:194],
                                  st[:, None, 0:1].to_broadcast((P, NTB, 1)))
            nc.vector.tensor_copy(
                v_sb[:, :, 66:129],
                st[:, None, 2:3].to_broadcast((P, NTB, 63)))
            nc.vector.tensor_copy(
                v_sb[:, :, 194:257],
                st[:, None, 2:3].to_broadcast((P, NTB, 63)))
            # causal 0/1 masks: mask_j[x, y] = 1 if y - x >= 128*j else 0
            nc.gpsimd.memset(mask_sb[:], 1.0)
            for j in range(4):
                nc.gpsimd.affine_select(
                    mask_sb[:, j, :], mask_sb[:, j, :],
                    pattern=[[1, QCW]],
                    compare_op=mybir.AluOpType.is_ge,
                    fill=0.0,
                    base=-128 * j,
                    channel_multiplier=-1,
                )

            # ---- qkT + V projections for one 512-token slice, split into
            # filler units emitted between attention pairs so the PE never
            # idles while ACT chews on exp ----
            def chain_unit(tj, ci):
                tjs = slice(tj * 512, (tj + 1) * 512)
                ps = ps_mm.tile([P, 512], F32, tag="mm")
                for kc in range(CKA):
                    nc.tensor.matmul(
                        ps[:],
                        wqk_sb[:, kc, ci * P:(ci + 1) * P],
                        xT_sb[:, kc, tjs],
                        start=(kc == 0), stop=(kc == CKA - 1),
                    )
                nc.vector.tensor_copy(qkT_sb[:, ci, tjs], ps[:])
                # tj0 relocs ride the (short) scalar queue: the first
                # attention pairs need them and sync is load-clogged
                reng = nc.scalar if tj == 0 else nc.sync
                reng.dma_start(qkT_sb[0:64, 3 + ci, tjs],
                               qkT_sb[64:128, ci, tjs])
                reng.dma_start(qkT_sb[64:128, 3 + ci, tjs],
                               qkT_sb[0:64, ci, tjs])

            def v_unit(tb):
                ps = ps_mm.tile([P, 512], F32, tag="mm")
                for kc in range(CK):
                    nc.tensor.matmul(
                        ps[:, :HPC * D],
                        xT_sb[:, kc, tb * P:(tb + 1) * P],
                        wv_sb[:, kc, :],
                        start=(kc == 0), stop=(kc == CK - 1),
                    )
                nc.vector.tensor_copy(v_sb[:, tb, 0:64], ps[:, 0:64])
                nc.vector.tensor_copy(v_sb[:, tb, 129:193],
                                      ps[:, 64:128])
                nc.vector.tensor_copy(v_sb[:, tb, 257:321],
                                      ps[:, 128:192])

            def project_tj(tj):
                for ci in range(3):
                    chain_unit(tj, ci)
                for tb in range(4 * tj, 4 * tj + 4):
                    v_unit(tb)

            # must: project(tj+1) units, required before attn(tj+1);
            # lazy: out-proj tiles, deferred into the ACT-bound attn(3)
            must, lazy = [], []

            def push_project_fillers(tj):
                for ci in range(3):
                    must.append(lambda tj=tj, ci=ci: chain_unit(tj, ci))
                for tb in range(4 * tj, 4 * tj + 4):
                    must.append(lambda tb=tb: v_unit(tb))

            def drain_fillers(k=1):
                for _ in range(k):
                    if must:
                        must.pop(0)()
                    elif lazy:
                        lazy.pop(0)()

            # ---- attention (kb pairs) and delayed normalization.
            # Normalization of head i is emitted during head i+1's matmul
            # loop so its DVE->gpsimd->DVE chain never stalls the PE. ----
            pend1, pend2 = [], []

            def norm_stage1(st8):
                qc, h, psy_t = st8
                v0, vw, srow, yrow = V_SLICE[h]
                den = small.tile([1, QCW], F32, tag="den")
                if srow == 0:
                    nc.vector.tensor_copy(den[:], psy_t[0:1, :])
                else:  # partition crossing: only ACT tolerates it
                    nc.scalar.copy(den[:], psy_t[srow:srow + 1, :])
                recf = small.tile([1, QCW], F32, tag="recf")
                nc.vector.reciprocal_approx_fast(recf[:], den[:])
                recip = small.tile([1, QCW], BF16, tag="recip")
                nc.vector.tensor_copy(recip[:], recf[:])
                psb = ps_mm.tile([P, QCW], F32, tag="mm", name="psb")
                nc.tensor.matmul(psb[:], ones1_sb[:], recip[:],
                                 start=True, stop=True)
                return (qc, h, psy_t, psb)

            def norm_stage2(st8):
                qc, h, psy_t, psb = st8
                q0 = qc * QCW
                v0, vw, srow, yrow = V_SLICE[h]
                yp, yci = Y_POS[h]
                bcf = small.tile([P, QCW], F32, tag="bcf")
                nc.vector.tensor_copy(
                    bcf[yrow:yrow + D, :], psb[yrow:yrow + D, :])
                nc.vector.tensor_mul(
                    yT_sb[yp:yp + D, yci, q0:q0 + QCW],
                    psy_t[yrow:yrow + D, :], bcf[yrow:yrow + D, :])

            def attn_qc(qc):
                q0 = qc * QCW
                npairs = 2 * qc + 2
                slots = HPC * (npairs - 1)
                lstride = max(1, -(-slots // (len(lazy) + 1)))
                slot = 0
                for h in range(HPC):
                    qpl, qcl = Q_LO[h]
                    qph, qch = Q_HI[h]
                    kpl, kcl = K_LO[h]
                    kph, kch = K_HI[h]
                    v0, vw, srow, yrow = V_SLICE[h]
                    psy_t = ps_y_pool.tile([P, QCW], F32, tag="yaug",
                                           name="psy")
                    psy = psy_t[0:vw, :]
                    for j in range(npairs):
                        kb0, kb1 = 2 * j, 2 * j + 1
                        n0 = max(0, kb0 * P - q0)
                        n1 = max(0, kb1 * P - q0)
                        pss = ps_s.tile([P, 1024], F32, tag="sc")
                        nc.tensor.matmul(
                            pss[:, n0:512],
                            qkT_sb[kpl:kpl + D, kcl, kb0 * P:(kb0 + 1) * P],
                            qkT_sb[qpl:qpl + D, qcl, q0 + n0:q0 + QCW],
                            start=True, stop=True,
                        )
                        if qc == 0:
                            # head of kernel: the hi relocation DMAs are
                            # still in flight; run both blocks on strip lo
                            nc.tensor.matmul(
                                pss[:, 512 + n1:1024],
                                qkT_sb[kpl:kpl + D, kcl,
                                       kb1 * P:(kb1 + 1) * P],
                                qkT_sb[qpl:qpl + D, qcl,
                                       q0 + n1:q0 + QCW],
                                start=True, stop=True,
                            )
                        else:
                            nc.tensor.matmul(
                                pss[:, 512 + n1:1024],
                                qkT_sb[kph:kph + D, kch,
                                       kb1 * P:(kb1 + 1) * P],
                                qkT_sb[qph:qph + D, qch,
                                       q0 + n1:q0 + QCW],
                                start=True, stop=True,
                            )
                        expT = work.tile([P, 1024], BF16, tag="expT")
                        if n0 == 0 and n1 == 0:
                            nc.scalar.activation(expT[:], pss[:], Exp,
                                                 bias=zb_sb[:])
                        else:
                            nc.scalar.activation(expT[:, n0:512],
                                                 pss[:, n0:512], Exp,
                                                 bias=zb_sb[:])
                            nc.scalar.activation(expT[:, 512 + n1:1024],
                                                 pss[:, 512 + n1:1024], Exp,
                                                 bias=zb_sb[:])
                        if kb0 * P >= q0:  # diagonal blocks
                            nc.vector.tensor_mul(
                                expT[:, n0:512], expT[:, n0:512],
                                mask_sb[:, kb0 - q0 // P, n0:])
                        if kb1 * P >= q0:
                            nc.vector.tensor_mul(
                                expT[:, 512 + n1:1024],
                                expT[:, 512 + n1:1024],
                                mask_sb[:, kb1 - q0 // P, n1:])
                        nc.tensor.matmul(
                            psy[:, n0:], v_sb[:, kb0, v0:v0 + vw],
                            expT[:, n0:512],
                            start=(j == 0), stop=False,
                        )
                        nc.tensor.matmul(
                            psy[:, n1:], v_sb[:, kb1, v0:v0 + vw],
                            expT[:, 512 + n1:1024],
                            start=False, stop=(j == npairs - 1),
                        )
                        if j == 0 and pend1:
                            pend2.append(norm_stage1(pend1.pop(0)))
                        if j == 1 and pend2:
                            norm_stage2(pend2.pop(0))
                        if j >= 1:
                            slot += 1
                            if must:
                                must.pop(0)()
                            if lazy and slot % lstride == 0:
                                lazy.pop(0)()
                    if debug_stages & 4:
                        pend1.append((qc, h, psy_t))

            def proj_tb(tb):
                osb = outp.tile([P, C], BF16, tag="osb")
                for half in range(2):
                    pso = ps_mm.tile([P, 512], F32, tag="mm",
                                     name="pso")[:, :NPH]
                    nc.tensor.matmul(
                        pso, yT_sb[:, 0, tb * P:(tb + 1) * P],
                        wp_sb[:, 0, half * NPH:(half + 1) * NPH],
                        start=True, stop=False)
                    nc.tensor.matmul(
                        pso, yT_sb[64:128, 1, tb * P:(tb + 1) * P],
                        wp_sb[64:128, 1, half * NPH:(half + 1) * NPH],
                        start=False, stop=True)
                    nc.vector.tensor_copy(
                        osb[:, half * NPH:(half + 1) * NPH], pso)
                (nc.sync if tb % 2 else nc.gpsimd).dma_start(
                    out[tb * P:(tb + 1) * P, :], osb[:])

            def proj_qc(qc):
                for tb in range(qc * QCW // P, (qc * QCW + QCW) // P):
                    proj_tb(tb)

            # pipeline: project(0) up front; during attn(tj) the filler
            # queue feeds project(tj+1) chains then out-proj(tj-1) tiles
            # into the PE gaps left by exp latency
            if debug_stages == 7:
                project_tj(0)
                for tj in range(T // 512):
                    if tj + 1 < NQC:
                        push_project_fillers(tj + 1)
                    if tj == 2:
                        for tb in range(0, 4):
                            lazy.append(lambda tb=tb: proj_tb(tb))
                    if tj == 3:
                        for tb in range(4, 10):
                            lazy.append(lambda tb=tb: proj_tb(tb))
                    attn_qc(tj)
                    drain_fillers(len(must) if tj + 1 < NQC
                                  else len(must) + len(lazy))
                # tail: the two held-back qc=2 out-proj tiles keep the PE
                # busy while the last head's norm chain drains
                proj_tb(4 * NQC - 6)
                proj_tb(4 * NQC - 5)
                while pend1:
                    pend2.append(norm_stage1(pend1.pop(0)))
                while pend2:
                    norm_stage2(pend2.pop(0))
                proj_qc(NQC - 1)
            else:
                for tj in range(T // 512):
                    project_tj(tj)
                    if debug_stages & 1:
                        attn_qc(tj)
                    if tj > 0 and debug_stages & 2:
                        proj_qc(tj - 1)
                if debug_stages & 1:
                    while pend1:
                        pend2.append(norm_stage1(pend1.pop(0)))
                    while pend2:
                        norm_stage2(pend2.pop(0))
                if debug_stages & 2:
                    proj_qc(NQC - 1)
            if debug_dump:
                nc.sync.dma_start(qkT_dump[:], qkT_sb[:])
                nc.sync.dma_start(v_dump[:], v_sb[:])
                if not (debug_stages & 1 and debug_stages & 4):
                    nc.gpsimd.memset(yT_sb[:], 0.0)
                nc.sync.dma_start(yT_dump[:], yT_sb[:])
                nc.sync.dma_start(mask_dump[:], mask_sb[:])
                nc.sync.dma_start(xT_dump[:], xT_sb[:])

    nc.compile()
    return nc


_NC_CACHE = {}


def _get_nc(T=2048, QCW=512):
    key = (T, QCW)
    if key not in _NC_CACHE:
        _NC_CACHE[key] = build_nc(T, QCW)
    return _NC_CACHE[key]


def build_in_maps(inputs):
    """Build the 8 per-core input dicts from full inputs."""
    x = np.asarray(inputs["x"], np.float32)
    W = np.asarray(inputs["W_attn"], np.float32)
    b = np.asarray(inputs["b_attn"], np.float32)
    W_proj = np.asarray(inputs["W_proj"], np.float32)
    in_maps = []
    for c in range(N_CORES):
        bi, g = divmod(c, 4)
        lo = g * (HPC * D)  # local head col offset within each of q/k/v
        qw = [W[:, lo + i * D:lo + (i + 1) * D] * 0.125 for i in range(HPC)]
        kw = [W[:, C + lo + i * D:C + lo + (i + 1) * D] for i in range(HPC)]
        qb = [b[lo + i * D:lo + (i + 1) * D] * 0.125 for i in range(HPC)]
        # chain order: [q0|q1], [k0|k1], [q2|k2]; row 768 = q bias row
        wqk = np.zeros((CKA * P, QK), np.float32)
        wqk[:C] = np.concatenate(
            [qw[0], qw[1], kw[0], kw[1], qw[2], kw[2]], axis=1)
        wqk[C, 0:64] = qb[0]
        wqk[C, 64:128] = qb[1]
        wqk[C, 256:320] = qb[2]
        wv = W[:, 2 * C + lo:2 * C + lo + HPC * D]
        # wp rows: chunk0 = [h0 | h1] (K=128); chunk1 = h2 (K=64)
        wp = np.ascontiguousarray(W_proj[lo:lo + HPC * D])
        in_maps.append({
            "xT": np.ascontiguousarray(x[bi].T).astype(BF),
            "wqk": np.ascontiguousarray(wqk).astype(BF),
            "wv": np.ascontiguousarray(wv).astype(BF),
            "wp": wp.astype(BF),
        })
    return in_maps


def postprocess(results, inputs):
    b_attn = np.asarray(inputs["b_attn"], np.float32)
    W_proj = np.asarray(inputs["W_proj"], np.float32)
    b_proj = np.asarray(inputs["b_proj"], np.float32)
    b_eff = b_proj + b_attn[2 * C:] @ W_proj
    T = results[0]["out"].shape[0]
    out = np.zeros((B, T, C), np.float32)
    for c in range(N_CORES):
        out[c // 4] += np.asarray(results[c]["out"], np.float32)
    out += b_eff
    return out


def kernel(x, W_attn, b_attn, W_proj, b_proj):
    inputs = dict(x=x, W_attn=W_attn, b_attn=b_attn,
                  W_proj=W_proj, b_proj=b_proj)
    T = np.asarray(x).shape[1]
    nc = _get_nc(T=T)
    in_maps = build_in_maps(inputs)
    res = bass_utils.run_bass_kernel_spmd(
        nc, in_maps, core_ids=list(range(N_CORES)))
    return postprocess(res.results, inputs)
